# revision 1
# baseline (speedup 1.0000x reference)
"""LinOSS layer Trainium2 kernel, v3.

Math (same closed form as v1): the per-state 2x2 recurrence has eigenvalues
e^{+-i theta}; the scanned state collapses to rank-2 modulated prefix sums

    u     = s * Bu                     (s folded into B on host)
    E     = cumsum(T1 * u);  F = cumsum(T2 * u)     per complex part
    x_t   = sin(t th) * E_t + cos(t th) * F_t
    T1    = gamma*cos + sin;  T2 = cos - gamma*sin

v3 hardware structure (one core; states sharded 32/core, time folded 4x
into partitions -> [128 = 4 chunks x 32 states, 2048] tiles):
  - input is transposed ON THE HOST (inpT [H, L]); all device loads are
    plain async DMA streams.  (XBAR dma transposes gang all 16 DMA engines
    and stall ~7us whenever any other transfer is in flight.)
  - T1/T2/sinT/cosT tables built exactly on the host (f64 -> bf16)
  - modulation = scalar_tensor_tensor with accum_out: per-partition chunk
    sums are free; carry offsets (Wm matmul) feed the scans as initial
    values -> no post-scan bias pass
  - DVE chain: 4 stt mods, 4 scans, 4 muls + 2 adds (demod) — everything
    else stays off DVE (Pool is ~4x slower and halves DVE when co-run)
  - projection split into two slabs the host sums: out0 = Ctr@x_r during
    scans 3-4, out1 = Cti@x_i in the tail (32 matmuls of 512 cols total,
    the PE minimum for this contraction)
  - dD term dropped; host adds input*D exactly
"""

import numpy as np

L, H, P = 8192, 128, 256
NCORES = 8
SLOC = P // NCORES          # states per core
FOLD = 4                    # time chunks folded into partitions
CL = L // FOLD              # 2048 free columns per partition row
NPART = FOLD * SLOC         # 128
JT = 512                    # j-tile width (psum bank)
NJT = CL // JT              # 4

_CACHE: dict = {}


def _build_bass(split_waits=True):
    import concourse.bass as bass
    import concourse.mybir as mybir
    import concourse.tile as tile

    dt = mybir.dt.float32
    bt = mybir.dt.bfloat16
    Alu = mybir.AluOpType

    nc = bass.Bass(
        trn_type="TRN2",
        target_bir_lowering=False,
        debug=False,
        num_devices=NCORES,
    )

    inpT_d = nc.dram_tensor("inpT", [H, L], bt, kind="ExternalInput").ap()
    Bt_d = nc.dram_tensor("Bt", [H, 2 * SLOC], bt, kind="ExternalInput").ap()
    T1_d = nc.dram_tensor("T1", [NPART, CL], bt, kind="ExternalInput").ap()
    T2_d = nc.dram_tensor("T2", [NPART, CL], bt, kind="ExternalInput").ap()
    sin_d = nc.dram_tensor("sinT", [NPART, CL], bt, kind="ExternalInput").ap()
    cos_d = nc.dram_tensor("cosT", [NPART, CL], bt, kind="ExternalInput").ap()
    Ctr_d = nc.dram_tensor("Ctr", [NPART, H], bt, kind="ExternalInput").ap()
    Cti_d = nc.dram_tensor("Cti", [NPART, H], bt, kind="ExternalInput").ap()
    Wm_d = nc.dram_tensor("Wm", [NPART, NPART], dt, kind="ExternalInput").ap()
    out0 = nc.dram_tensor("out0", [H, L], bt, kind="ExternalOutput").ap()
    out1 = nc.dram_tensor("out1", [H, L], bt, kind="ExternalOutput").ap()

    with tile.TileContext(nc) as tc:
        cpool = tc.alloc_tile_pool(name="const", bufs=1)
        big = tc.alloc_tile_pool(name="big", bufs=1)
        stage = tc.alloc_tile_pool(name="stage", bufs=6)
        pbu_i_pool = tc.alloc_tile_pool(name="pbu_i", bufs=1, space="PSUM")
        pbu_r_pool = tc.alloc_tile_pool(name="pbu_r", bufs=1, space="PSUM")

        # ---- loads, earliest-needed first; all plain async streams ----
        Bt = cpool.tile([H, 2 * SLOC], bt)
        nc.sync.dma_start(out=Bt[:], in_=Bt_d)
        inpT = big.tile([128, L], bt, tag="inpT")
        for p8 in range(8):
            nc.sync.dma_start(
                out=inpT[:, p8 * (L // 8):(p8 + 1) * (L // 8)],
                in_=inpT_d[:, p8 * (L // 8):(p8 + 1) * (L // 8)],
            )
        # T1/T2 deferred until inpT is mostly in (DMA bw is fair-shared;
        # a WAR dep on a dummy slot is the only reliable deferral)
        T1d_t = big.tile([NPART, CL], bt, tag="T1")
        T2d_t = big.tile([NPART, CL], bt, tag="T2")
        gate1 = cpool.tile([1, 8], dt)
        nc.gpsimd.memset(T1d_t[0:1, 0:8], 0.0)
        nc.gpsimd.memset(T2d_t[0:1, 0:8], 0.0)
        nc.gpsimd.tensor_tensor(
            gate1[:], T1d_t[0:1, 0:8], inpT[0:1, 1 * (L // 8):1 * (L // 8) + 8],
            mybir.AluOpType.add)
        nc.gpsimd.tensor_tensor(
            gate1[:], T2d_t[0:1, 0:8], inpT[0:1, 1 * (L // 8):1 * (L // 8) + 8],
            mybir.AluOpType.add)
        T1 = big.tile([NPART, CL], bt, tag="T1")
        T2 = big.tile([NPART, CL], bt, tag="T2")
        for tt_, td_ in ((T1, T1_d), (T2, T2_d)):
            for hh in range(2):
                nc.sync.dma_start(
                    out=tt_[:, hh * (CL // 2):(hh + 1) * (CL // 2)],
                    in_=td_[:, hh * (CL // 2):(hh + 1) * (CL // 2)],
                )
        Ctr = cpool.tile([NPART, H], bt)
        Cti = cpool.tile([NPART, H], bt)
        Wm = cpool.tile([NPART, NPART], dt)
        nc.sync.dma_start(out=Ctr[:], in_=Ctr_d)
        nc.sync.dma_start(out=Cti[:], in_=Cti_d)
        nc.sync.dma_start(out=Wm[:], in_=Wm_d)

        ones = cpool.tile([NPART, CL], bt)
        nc.vector.memset(ones[:], 1.0)

        # ---- Bu matmuls into full-width PSUM; mods read PSUM directly
        # (stt has no 2x mode regardless, so the u evac would buy nothing)
        pbu_r = pbu_r_pool.tile([NPART, CL], dt, tag="bu_r")
        pbu_i = pbu_i_pool.tile([NPART, CL], dt, tag="bu_i")
        for pbu, bs in ((pbu_r, slice(0, SLOC)),
                        (pbu_i, slice(SLOC, 2 * SLOC))):
            for jt in range(NJT):
                for c in range(FOLD):
                    rhs = inpT[:, c * CL + jt * JT: c * CL + (jt + 1) * JT]
                    ps = slice(c * SLOC, (c + 1) * SLOC)
                    nc.tensor.matmul(
                        pbu[ps, jt * JT:(jt + 1) * JT], Bt[:, bs], rhs,
                        start=True, stop=True,
                        tile_position=(0, c * SLOC),
                    )

        # ---- deferred table loads: DMA bandwidth is fair-shared across all
        # in-flight transfers, so these 1.2MiB must not start until the
        # critical inpT/T1/T2 transfers are done.  A Pool op depending on
        # u_r gates the SWDGE issues. ----
        # sin/cos wave 3: gated on T2's arrival the same way
        sind_t = big.tile([NPART, CL], bt, tag="sinT")
        cosd_t = big.tile([NPART, CL], bt, tag="cosT")
        gatet = cpool.tile([1, 8], dt)
        nc.gpsimd.memset(sind_t[0:1, 0:8], 0.0)
        nc.gpsimd.memset(cosd_t[0:1, 0:8], 0.0)
        gsrc = inpT[0:1, 6 * (L // 8):6 * (L // 8) + 8]
        nc.gpsimd.tensor_tensor(
            gatet[:], sind_t[0:1, 0:8], gsrc, mybir.AluOpType.add)
        nc.gpsimd.tensor_tensor(
            gatet[:], cosd_t[0:1, 0:8], gsrc, mybir.AluOpType.add)
        sinT = big.tile([NPART, CL], bt, tag="sinT")
        cosT = big.tile([NPART, CL], bt, tag="cosT")
        # issue from two different engines so the transfers (and their
        # modeled arrivals) run in parallel: m_b needs cosT before scan-Ei
        nc.gpsimd.dma_start(out=sinT[:], in_=sin_d)
        nc.scalar.dma_start(out=cosT[:], in_=cos_d)

        # ---- modulation w/ fused chunk sums (DVE stt) ----
        A = cpool.tile([NPART, 4], dt)
        Y1r = big.tile([NPART, CL], bt, tag="Y1r")
        Y2r = big.tile([NPART, CL], bt, tag="Y2r")
        Y1i = big.tile([NPART, CL], bt, tag="Y1i")
        Y2i = big.tile([NPART, CL], bt, tag="Y2i")
        # T1-based mods first: T2 lands ~2.3us after T1, and this order
        # fills that DVE gap with Y1i instead of idling
        mods = [(Y1r, T1, pbu_r, 0), (Y1i, T1, pbu_i, 2),
                (Y2r, T2, pbu_r, 1), (Y2i, T2, pbu_i, 3)]
        offs = cpool.tile([NPART, 4], dt)
        for k, (Y, T, u, ai) in enumerate(mods):
            # modulation with fused chunk-sum accumulation (DVE stt)
            nc.vector.scalar_tensor_tensor(
                Y[:], T[:], 1.0, u[:], Alu.mult, Alu.mult,
                accum_out=A[:, ai:ai + 1],
            )
            if k == 2:
                # r-pair offsets (Y1r, Y2r read pbu_r; both done now, so
                # its corner is WAR-free for the matmul bounce); the r-scans
                # read this psum region directly as their initial value,
                # skipping the ACT-copy latency on the critical chain
                nc.tensor.matmul(
                    pbu_r[:, 0:2], Wm[:], A[:, 0:2], start=True, stop=True)
            elif k == 3:
                nc.tensor.matmul(
                    pbu_i[:, 0:2], Wm[:], A[:, 2:4], start=True, stop=True)
                nc.scalar.copy(offs[:, 2:4], pbu_i[:, 0:2])

        pbu_r_pool.release()
        pbu_i_pool.release()
        po = tc.alloc_tile_pool(name="po", bufs=4, space="PSUM")

        # ---- scans (initial = carry offsets) + demod (all DVE) ----
        Er = big.tile([NPART, CL], bt, tag="Er")
        Fr = big.tile([NPART, CL], bt, tag="Fr")
        Ei = big.tile([NPART, CL], bt, tag="Ei")
        Fi = big.tile([NPART, CL], bt, tag="Fi")
        m_a = big.tile([NPART, CL], bt, tag="m_a")
        m_b = big.tile([NPART, CL], bt, tag="m_b")
        m_c = big.tile([NPART, CL], bt, tag="m_c")
        m_d = big.tile([NPART, CL], bt, tag="m_d")
        x_r = big.tile([NPART, CL], bt, tag="x_r")
        x_i = big.tile([NPART, CL], bt, tag="x_i")

        def scan(out, y, ai):
            ini = pbu_r[:, ai:ai + 1] if ai < 2 else offs[:, ai:ai + 1]
            bass.BassGpSimd.tensor_tensor_scan(
                nc.vector, out[:], ones[:], y[:], ini,
                Alu.mult, Alu.add,
            )

        scan(Er, Y1r, 0)
        scan(Fr, Y2r, 1)
        with tc.high_priority():
            nc.vector.tensor_mul(m_a[:], Er[:], sinT[:])
            nc.vector.tensor_mul(m_b[:], Fr[:], cosT[:])
            nc.vector.tensor_add(x_r[:], m_a[:], m_b[:])
        scan(Ei, Y1i, 2)
        with tc.high_priority():
            nc.vector.tensor_mul(m_c[:], Ei[:], sinT[:])
        scan(Fi, Y2i, 3)
        with tc.high_priority():
            nc.vector.tensor_mul(m_d[:], Fi[:], cosT[:])
            nc.vector.tensor_add(x_i[:], m_c[:], m_d[:])

        # ---- projection slabs: out0 = Ctr@x_r (under scans 3-4),
        #      out1 = Cti@x_i (tail); host sums the slabs ----
        for slab, (Wt, x, outd) in enumerate(((Ctr, x_r, out0),
                                              (Cti, x_i, out1))):
            loop = ([(c, h2) for c in range(FOLD) for h2 in range(2)]
                    if slab == 0 else
                    [(c, h2) for h2 in range(2) for c in range(FOLD)])
            for c, h2 in loop:
                ps = slice(c * SLOC, (c + 1) * SLOC)
                if True:
                    pt = po.tile([128, 2 * JT], dt, tag="po")
                    for jh in range(2):
                        jt = 2 * h2 + jh
                        js = slice(jt * JT, (jt + 1) * JT)
                        nc.tensor.matmul(
                            pt[:, jh * JT:(jh + 1) * JT], Wt[ps, :],
                            x[ps, js], start=True, stop=True,
                            tile_position=(c * SLOC, 0),
                        )
                    st = stage.tile([128, 2 * JT], bt, tag="st")
                    # slab0 evacs run under the scans (ACT); slab1 evacs
                    # land in the tail, where DVE is free — alternate
                    if slab == 1 and (c * 2 + h2) % 2 == 0:
                        nc.vector.tensor_copy(st[:], pt[:])
                    else:
                        nc.scalar.copy(st[:], pt[:])
                    nc.sync.dma_start(
                        out=outd[:, c * CL + h2 * 2 * JT:
                                 c * CL + (h2 + 1) * 2 * JT],
                        in_=st[:],
                    )
        for p in (po, stage, big, cpool):
            p.release()
    if split_waits:
        _split_matmul_waits(nc, mybir)
    return nc


def _split_matmul_waits(nc, mybir):
    """Hardware instruction structs fit a limited number of embedded sync
    waits; move extra waits onto an inserted same-queue no-op."""
    caps = {"InstMatmult": 1}
    skip = {"InstNoOp", "InstAllEngineBarrier", "InstSync"}
    k = 0
    for bb in nc.main_func.blocks:
        insts = bb.instructions
        i = 0
        while i < len(insts):
            ins = insts[i]
            tn = type(ins).__name__
            if tn not in skip and ins.sync_info is not None:
                cap = caps.get(tn, 1)
                w = list(ins.sync_info.on_wait or [])
                if len(w) > cap:
                    for wj in w[:-cap]:
                        nop = mybir.InstNoOp(
                            name=f"I-mmdep-{k}",
                            engine=ins.engine,
                            ins=[],
                            outs=[],
                            sync_info=mybir.SyncInfo(
                                on_wait=[wj], on_update=[]
                            ),
                        )
                        k += 1
                        insts.insert(i, nop)
                        i += 1
                    ins.sync_info = mybir.SyncInfo(
                        on_wait=w[-cap:], on_update=ins.sync_info.on_update
                    )
            i += 1


def _host_prep(inputs):
    import ml_dtypes
    bf16 = ml_dtypes.bfloat16
    f32 = np.float32

    inp32 = np.asarray(inputs["input_sequence"], np.float32)
    inpT = np.ascontiguousarray(inp32.T.astype(bf16))
    A = np.maximum(np.asarray(inputs["A_diag_raw"], np.float64), 0.0)
    s = 1.0 / (1.0 + np.exp(-np.asarray(inputs["steps_raw"], np.float64)))
    Br = np.asarray(inputs["B_real"], np.float64)
    Bi = np.asarray(inputs["B_img"], np.float64)
    Cr = np.asarray(inputs["C_real"], np.float64)
    Ci = np.asarray(inputs["C_img"], np.float64)

    costh = 1.0 - s * s * A / 2.0
    sinth = np.sqrt(np.maximum(1.0 - costh * costh, 1e-300))
    theta = np.arctan2(sinth, costh)
    gamma = (s - s * s * A / 2.0) / sinth

    q = np.arange(NPART)
    Wm = ((q[:, None] % SLOC == q[None, :] % SLOC)
          & (q[:, None] // SLOC < q[None, :] // SLOC)).astype(f32)

    tvec = np.arange(CL, dtype=np.float64)
    twopi = 2.0 * np.pi

    in_maps = []
    for k in range(NCORES):
        sl = slice(k * SLOC, (k + 1) * SLOC)
        th = theta[sl]
        gm = gamma[sl]
        Bt = np.empty((H, 2 * SLOC), bf16)
        Bt[:, 0:SLOC] = (s[sl, None] * Br[sl]).T.astype(bf16)
        Bt[:, SLOC:] = (s[sl, None] * Bi[sl]).T.astype(bf16)
        Ctr = np.tile(Cr[:, sl].T, (FOLD, 1)).astype(bf16)
        Cti = np.tile(-Ci[:, sl].T, (FOLD, 1)).astype(bf16)

        # tables per partition q = c*SLOC + s at global time t = c*CL + j
        ang = np.empty((NPART, CL), np.float64)
        for c in range(FOLD):
            ang[c * SLOC:(c + 1) * SLOC] = np.mod(
                (c * CL + tvec)[None, :] * th[:, None], twopi)
        sinA = np.sin(ang)
        cosA = np.cos(ang)
        gq = np.tile(gm, FOLD)[:, None]
        T1 = (gq * cosA + sinA).astype(bf16)
        T2 = (cosA - gq * sinA).astype(bf16)

        in_maps.append({
            "inpT": inpT,
            "Bt": Bt,
            "T1": np.ascontiguousarray(T1),
            "T2": np.ascontiguousarray(T2),
            "sinT": np.ascontiguousarray(sinA.astype(bf16)),
            "cosT": np.ascontiguousarray(cosA.astype(bf16)),
            "Ctr": Ctr,
            "Cti": Cti,
            "Wm": Wm,
        })
    return in_maps


LAST_RESULTS = None


def kernel(**inputs) -> np.ndarray:
    global LAST_RESULTS
    from concourse.bass_utils import run_bass_kernel_spmd

    if "nc" not in _CACHE:
        _CACHE["nc"] = _build_bass()
    nc = _CACHE["nc"]

    in_maps = _host_prep(inputs)
    res = run_bass_kernel_spmd(nc, in_maps, core_ids=list(range(NCORES)))
    LAST_RESULTS = res
    part = np.zeros((H, L), np.float32)
    for r in res.results:
        part += np.asarray(r["out0"], np.float32)
        part += np.asarray(r["out1"], np.float32)
    out = np.ascontiguousarray(part.T)
    out += (np.asarray(inputs["input_sequence"], np.float32)
            * np.asarray(inputs["D"], np.float32)[None, :])
    return out



# revision 6
# speedup vs baseline: 1.0077x; 1.0077x over previous
"""LinOSS layer Trainium2 kernel, v4.

Math (same closed form as v3): the per-state 2x2 recurrence has eigenvalues
e^{+-i theta}; the scanned state collapses to rank-2 modulated prefix sums

    u     = s * Bu                     (s folded into B on host)
    E     = cumsum(T1 * u);  F = cumsum(T2 * u)     per complex part
    x_t   = sin(t th) * E_t + cos(t th) * F_t
    T1    = gamma*cos + sin;  T2 = cos - gamma*sin

v4 changes vs v3 (keeps the 128 = 4 time-chunks x 32 states partition fold):
  - EVEN/ODD TIME SPLIT, done on the host: inpT columns are permuted to
    [chunk c | evens 1024 | odds 1024]; all tables pre-split to match.
    The DVE scan (2 cycles/col, no perf modes) then only runs over the
    1024 pair-sums P_j = y_{2j} + y_{2j+1}:
        S_{2j+1} = seed + cumsum(P)_j          (scan, half length)
        S_{2j}   = S_{2j+1} - y_{2j+1}         (aligned 2x tensor_tensor sub)
    halving the dominant scan cost.
  - u is evacuated from PSUM to bf16 SBUF by the (idle) ACT engine, so the
    modulations run as all-bf16 tensor_tensor mults in DVE 2x mode instead
    of 1x stt-on-PSUM.
  - the pair-sum runs as an stt with accum_out, which yields the per-chunk
    sums for the carry (Wm matmul -> scan initial values) for free.
  - projection/evac/output structure as v3 (two slabs out0/out1, host sums,
    host also un-permutes the even/odd column blocks).
"""

import numpy as np

L, H, P = 8192, 128, 256
NCORES = 8
SLOC = P // NCORES          # states per core
FOLD = 4                    # time chunks folded into partitions
CL = L // FOLD              # 2048 free columns per partition row
HCL = CL // 2               # 1024 columns after even/odd split
NPART = FOLD * SLOC         # 128
JT = 512                    # j-tile width (psum bank)

_CACHE: dict = {}


def _build_bass(split_waits=True):
    import concourse.bass as bass
    import concourse.mybir as mybir
    import concourse.tile as tile

    dt = mybir.dt.float32
    bt = mybir.dt.bfloat16
    Alu = mybir.AluOpType

    nc = bass.Bass(
        trn_type="TRN2",
        target_bir_lowering=False,
        debug=False,
        num_devices=NCORES,
    )

    inpT_d = nc.dram_tensor("inpT", [H, L], bt, kind="ExternalInput").ap()
    Bt_d = nc.dram_tensor("Bt", [H, 2 * SLOC], bt, kind="ExternalInput").ap()
    tab_names = ["T1e", "T1o", "T2e", "T2o", "sinTe", "sinTo", "cosTe", "cosTo"]
    tab_d = {n: nc.dram_tensor(n, [NPART, HCL], bt, kind="ExternalInput").ap()
             for n in tab_names}
    Ctr_d = nc.dram_tensor("Ctr", [NPART, H], bt, kind="ExternalInput").ap()
    Cti_d = nc.dram_tensor("Cti", [NPART, H], bt, kind="ExternalInput").ap()
    Wm_d = nc.dram_tensor("Wm", [NPART, NPART], dt, kind="ExternalInput").ap()
    out0 = nc.dram_tensor("out0", [H, L], bt, kind="ExternalOutput").ap()
    out1 = nc.dram_tensor("out1", [H, L], bt, kind="ExternalOutput").ap()

    with tile.TileContext(nc) as tc:
        cpool = tc.alloc_tile_pool(name="const", bufs=1)
        big = tc.alloc_tile_pool(name="big", bufs=1)
        stage = tc.alloc_tile_pool(name="stage", bufs=6)
        pbu_i_pool = tc.alloc_tile_pool(name="pbu_i", bufs=1, space="PSUM")
        pbu_r_pool = tc.alloc_tile_pool(name="pbu_r", bufs=1, space="PSUM")

        # ---- loads, earliest-needed first; all plain async streams ----
        Bt = cpool.tile([H, 2 * SLOC], bt)
        nc.sync.dma_start(out=Bt[:], in_=Bt_d)
        # inpT in host-permuted (chunk, even/odd, 1024) column blocks.
        # Load the 4 even blocks first, then the 4 odd blocks, so the
        # even-side Bu matmuls + evacs + mods can run during the odd loads.
        inpT = big.tile([128, L], bt, tag="inpT")
        KB = L // 8  # 1024-col dma blocks
        for k8 in (0, 2, 4, 6, 1, 3, 5, 7):
            nc.sync.dma_start(
                out=inpT[:, k8 * KB:(k8 + 1) * KB],
                in_=inpT_d[:, k8 * KB:(k8 + 1) * KB],
            )
        Wm = cpool.tile([NPART, NPART], dt)
        nc.sync.dma_start(out=Wm[:], in_=Wm_d)

        # deferred table loads: DMA bandwidth is fair-shared across all
        # in-flight transfers, so these must not start until the critical
        # inpT transfers are mostly done.  A gpsimd op reading an inpT
        # chunk and writing a dummy slot of the target tile creates the
        # WAR dep that reliably defers the SWDGE issue (v3 trick).
        tabs = {}

        def gated_load(names, gate_k8):
            gsrc = inpT[0:1, gate_k8 * KB:gate_k8 * KB + 8]
            for n in names:
                dummy = big.tile([NPART, HCL], bt, tag=n)
                gd = cpool.tile([1, 8], dt, tag=f"gate_{n}")
                nc.gpsimd.memset(dummy[0:1, 0:8], 0.0)
                nc.gpsimd.tensor_tensor(gd[:], dummy[0:1, 0:8], gsrc, Alu.add)
                real = big.tile([NPART, HCL], bt, tag=n)
                tabs[n] = real
                nc.sync.dma_start(out=real[:], in_=tab_d[n])

        gated_load(["T1e", "T2e"], 4)      # after 3rd even block
        gated_load(["T1o", "T2o"], 3)      # after 2nd odd block
        gated_load(["sinTe", "cosTe", "sinTo", "cosTo"], 7)  # after last block
        Ctr = cpool.tile([NPART, H], bt)
        Cti = cpool.tile([NPART, H], bt)
        nc.sync.dma_start(out=Ctr[:], in_=Ctr_d)
        nc.sync.dma_start(out=Cti[:], in_=Cti_d)

        ones = cpool.tile([NPART, HCL], bt)
        nc.vector.memset(ones[:], 1.0)

        # ---- Bu matmuls into full-width PSUM; even halves first ----
        pbu_r = pbu_r_pool.tile([NPART, CL], dt, tag="bu_r")
        pbu_i = pbu_i_pool.tile([NPART, CL], dt, tag="bu_i")
        bs_r = slice(0, SLOC)
        bs_i = slice(SLOC, 2 * SLOC)
        for jt in (0, 1, 2, 3):
            for pbu, bs in ((pbu_r, bs_r), (pbu_i, bs_i)):
                for c in range(FOLD):
                    rhs = inpT[:, c * CL + jt * JT: c * CL + (jt + 1) * JT]
                    ps = slice(c * SLOC, (c + 1) * SLOC)
                    nc.tensor.matmul(
                        pbu[ps, jt * JT:(jt + 1) * JT], Bt[:, bs], rhs,
                        start=True, stop=True,
                        tile_position=(0, c * SLOC),
                    )

        # ---- ACT evac of u to bf16 SBUF (so mods run in DVE 2x mode) ----
        ue_r = big.tile([NPART, HCL], bt, tag="ue_r")
        ue_i = big.tile([NPART, HCL], bt, tag="ue_i")
        uo_r = big.tile([NPART, HCL], bt, tag="uo_r")
        uo_i = big.tile([NPART, HCL], bt, tag="uo_i")
        nc.scalar.copy(ue_r[:], pbu_r[:, 0:HCL])
        nc.scalar.copy(ue_i[:], pbu_i[:, 0:HCL])
        nc.scalar.copy(uo_r[:], pbu_r[:, HCL:CL])
        nc.scalar.copy(uo_i[:], pbu_i[:, HCL:CL])

        # ---- modulations (tt 2x) + pair-sums (stt w/ accum) ----
        A = cpool.tile([NPART, 4], dt)
        Y = {}
        Pq = {}
        # (quantity, table-even, table-odd, u-even, u-odd, accum col)
        quants = [("1r", "T1e", "T1o", ue_r, uo_r, 0),
                  ("2r", "T2e", "T2o", ue_r, uo_r, 1),
                  ("1i", "T1e", "T1o", ue_i, uo_i, 2),
                  ("2i", "T2e", "T2o", ue_i, uo_i, 3)]
        offs = cpool.tile([NPART, 4], dt)
        # even-side mods first: their inputs land while the odd DMA blocks
        # are still streaming, and the DVE queue drains in order
        for q, te, to, ue, uo, ai in quants:
            Ye = big.tile([NPART, HCL], bt, tag=f"Ye{q}")
            nc.vector.tensor_mul(Ye[:], tabs[te][:], ue[:])
            Y[q] = [Ye, None]
        for q, te, to, ue, uo, ai in quants:
            Yo = big.tile([NPART, HCL], bt, tag=f"Yo{q}")
            Pt = big.tile([NPART, HCL], bt, tag=f"P{q}")
            nc.vector.tensor_mul(Yo[:], tabs[to][:], uo[:])
            nc.vector.scalar_tensor_tensor(
                Pt[:], Y[q][0][:], 1.0, Yo[:], Alu.mult, Alu.add,
                accum_out=A[:, ai:ai + 1],
            )
            Y[q][1] = Yo
            Pq[q] = Pt
            if ai == 1:
                # r-pair carries -> psum corner (WAR-free after ue_r evac);
                # the r-scans read this psum region directly as initial
                nc.tensor.matmul(
                    pbu_r[:, 0:2], Wm[:], A[:, 0:2], start=True, stop=True)
            elif ai == 3:
                nc.tensor.matmul(
                    pbu_i[:, 0:2], Wm[:], A[:, 2:4], start=True, stop=True)
                nc.scalar.copy(offs[:, 2:4], pbu_i[:, 0:2])

        # ---- scans over pair-sums (half length) + fixup + demod ----
        So = {}
        Se = {}

        def scan(q, ai):
            out = big.tile([NPART, HCL], bt, tag=f"So{q}")
            ini = pbu_r[:, ai:ai + 1] if ai < 2 else offs[:, ai:ai + 1]
            bass.BassGpSimd.tensor_tensor_scan(
                nc.vector, out[:], ones[:], Pq[q][:], ini,
                Alu.mult, Alu.add,
            )
            So[q] = out

        def fix(q):
            # S_even = S_odd - y_odd  (aligned, 2x)
            out = big.tile([NPART, HCL], bt, tag=f"Se{q}")
            nc.vector.tensor_sub(out[:], So[q][:], Y[q][1][:])
            Se[q] = out

        x_re = big.tile([NPART, HCL], bt, tag="x_re")
        x_ro = big.tile([NPART, HCL], bt, tag="x_ro")
        x_ie = big.tile([NPART, HCL], bt, tag="x_ie")
        x_io = big.tile([NPART, HCL], bt, tag="x_io")
        m_a = big.tile([NPART, HCL], bt, tag="m_a")
        m_b = big.tile([NPART, HCL], bt, tag="m_b")

        def demod(xo, xe, qE, qF):
            # x_odd = sinTo*S_odd(E) + cosTo*S_odd(F); even likewise
            with tc.high_priority():
                nc.vector.tensor_mul(m_a[:], So[qE][:], tabs["sinTo"][:])
                nc.vector.tensor_mul(m_b[:], So[qF][:], tabs["cosTo"][:])
                nc.vector.tensor_add(xo[:], m_a[:], m_b[:])
                fix(qE)
                fix(qF)
                nc.vector.tensor_mul(m_a[:], Se[qE][:], tabs["sinTe"][:])
                nc.vector.tensor_mul(m_b[:], Se[qF][:], tabs["cosTe"][:])
                nc.vector.tensor_add(xe[:], m_a[:], m_b[:])

        scan("1r", 0)
        scan("2r", 1)
        demod(x_ro, x_re, "1r", "2r")
        scan("1i", 2)
        scan("2i", 3)
        demod(x_io, x_ie, "1i", "2i")

        pbu_r_pool.release()
        pbu_i_pool.release()
        po = tc.alloc_tile_pool(name="po", bufs=4, space="PSUM")

        # ---- projection slabs: out0 = Ctr@x_r (under i scans/demods),
        #      out1 = Cti@x_i (tail); host sums the slabs.
        # column blocks: (chunk c, half h in {e,o}, 1024) matching inpT ----
        for slab, (Wt, xe, xo, outd) in enumerate(
                ((Ctr, x_re, x_ro, out0), (Cti, x_ie, x_io, out1))):
            for c in range(FOLD):
                # odd half first: x_odd comes straight off the scans, x_even
                # needs the fixup+demod that follow
                for h, x in ((1, xo), (0, xe)):
                    ps = slice(c * SLOC, (c + 1) * SLOC)
                    pt = po.tile([128, 2 * JT], dt, tag="po")
                    for jh in range(2):
                        js = slice(jh * JT, (jh + 1) * JT)
                        nc.tensor.matmul(
                            pt[:, js], Wt[ps, :], x[ps, js],
                            start=True, stop=True,
                            tile_position=(c * SLOC, 0),
                        )
                    st = stage.tile([128, 2 * JT], bt, tag="st")
                    # slab0 evacs run under the i-chain (ACT); slab1 evacs
                    # land in the tail, where DVE is free - alternate
                    if slab == 1 and (c * 2 + h) % 2 == 0:
                        nc.vector.tensor_copy(st[:], pt[:])
                    else:
                        nc.scalar.copy(st[:], pt[:])
                    nc.sync.dma_start(
                        out=outd[:, c * CL + h * HCL: c * CL + (h + 1) * HCL],
                        in_=st[:],
                    )
        for p in (po, stage, big, cpool):
            p.release()
    if split_waits:
        _split_matmul_waits(nc, mybir)
    return nc


def _split_matmul_waits(nc, mybir):
    """Hardware instruction structs fit a limited number of embedded sync
    waits; move extra waits onto an inserted same-queue no-op."""
    caps = {"InstMatmult": 1}
    skip = {"InstNoOp", "InstAllEngineBarrier", "InstSync"}
    k = 0
    for bb in nc.main_func.blocks:
        insts = bb.instructions
        i = 0
        while i < len(insts):
            ins = insts[i]
            tn = type(ins).__name__
            if tn not in skip and ins.sync_info is not None:
                cap = caps.get(tn, 1)
                w = list(ins.sync_info.on_wait or [])
                if len(w) > cap:
                    for wj in w[:-cap]:
                        nop = mybir.InstNoOp(
                            name=f"I-mmdep-{k}",
                            engine=ins.engine,
                            ins=[],
                            outs=[],
                            sync_info=mybir.SyncInfo(
                                on_wait=[wj], on_update=[]
                            ),
                        )
                        k += 1
                        insts.insert(i, nop)
                        i += 1
                    ins.sync_info = mybir.SyncInfo(
                        on_wait=w[-cap:], on_update=ins.sync_info.on_update
                    )
            i += 1


def _eo_permute(a):
    """[rows, 2048-per-chunk...] -> per 2048-chunk [evens 1024 | odds 1024]."""
    r, n = a.shape
    nch = n // CL
    return np.ascontiguousarray(
        a.reshape(r, nch, CL // 2, 2).transpose(0, 1, 3, 2).reshape(r, n))


def _eo_unpermute(a):
    r, n = a.shape
    nch = n // CL
    return np.ascontiguousarray(
        a.reshape(r, nch, 2, CL // 2).transpose(0, 1, 3, 2).reshape(r, n))


def _host_prep(inputs):
    import ml_dtypes
    bf16 = ml_dtypes.bfloat16
    f32 = np.float32

    inp32 = np.asarray(inputs["input_sequence"], np.float32)
    inpT = _eo_permute(np.ascontiguousarray(inp32.T)).astype(bf16)
    A = np.maximum(np.asarray(inputs["A_diag_raw"], np.float64), 0.0)
    s = 1.0 / (1.0 + np.exp(-np.asarray(inputs["steps_raw"], np.float64)))
    Br = np.asarray(inputs["B_real"], np.float64)
    Bi = np.asarray(inputs["B_img"], np.float64)
    Cr = np.asarray(inputs["C_real"], np.float64)
    Ci = np.asarray(inputs["C_img"], np.float64)

    costh = 1.0 - s * s * A / 2.0
    sinth = np.sqrt(np.maximum(1.0 - costh * costh, 1e-300))
    theta = np.arctan2(sinth, costh)
    gamma = (s - s * s * A / 2.0) / sinth

    q = np.arange(NPART)
    Wm = ((q[:, None] % SLOC == q[None, :] % SLOC)
          & (q[:, None] // SLOC < q[None, :] // SLOC)).astype(f32)

    tvec = np.arange(CL, dtype=np.float64)
    twopi = 2.0 * np.pi

    in_maps = []
    for k in range(NCORES):
        sl = slice(k * SLOC, (k + 1) * SLOC)
        th = theta[sl]
        gm = gamma[sl]
        Bt = np.empty((H, 2 * SLOC), bf16)
        Bt[:, 0:SLOC] = (s[sl, None] * Br[sl]).T.astype(bf16)
        Bt[:, SLOC:] = (s[sl, None] * Bi[sl]).T.astype(bf16)
        Ctr = np.tile(Cr[:, sl].T, (FOLD, 1)).astype(bf16)
        Cti = np.tile(-Ci[:, sl].T, (FOLD, 1)).astype(bf16)

        # tables per partition q = c*SLOC + s at global time t = c*CL + j
        ang = np.empty((NPART, CL), np.float64)
        for c in range(FOLD):
            ang[c * SLOC:(c + 1) * SLOC] = np.mod(
                (c * CL + tvec)[None, :] * th[:, None], twopi)
        sinA = np.sin(ang)
        cosA = np.cos(ang)
        gq = np.tile(gm, FOLD)[:, None]
        T1 = gq * cosA + sinA
        T2 = cosA - gq * sinA

        m = {"inpT": inpT, "Bt": Bt, "Ctr": Ctr, "Cti": Cti, "Wm": Wm}
        for nm, tb in (("T1", T1), ("T2", T2), ("sinT", sinA), ("cosT", cosA)):
            m[nm + "e"] = np.ascontiguousarray(tb[:, 0::2]).astype(bf16)
            m[nm + "o"] = np.ascontiguousarray(tb[:, 1::2]).astype(bf16)
        in_maps.append(m)
    return in_maps


LAST_RESULTS = None


def kernel(**inputs) -> np.ndarray:
    global LAST_RESULTS
    from concourse.bass_utils import run_bass_kernel_spmd

    if "nc" not in _CACHE:
        _CACHE["nc"] = _build_bass()
    nc = _CACHE["nc"]

    in_maps = _host_prep(inputs)
    res = run_bass_kernel_spmd(nc, in_maps, core_ids=list(range(NCORES)))
    LAST_RESULTS = res
    part = np.zeros((H, L), np.float32)
    for r in res.results:
        part += np.asarray(r["out0"], np.float32)
        part += np.asarray(r["out1"], np.float32)
    out = np.ascontiguousarray(_eo_unpermute(part).T)
    out += (np.asarray(inputs["input_sequence"], np.float32)
            * np.asarray(inputs["D"], np.float32)[None, :])
    return out


# revision 10
# speedup vs baseline: 1.0649x; 1.0567x over previous
"""LinOSS layer Trainium2 kernel, v4.2.

Math (same closed form as v3): the per-state 2x2 recurrence has eigenvalues
e^{+-i theta}; the scanned state collapses to rank-2 modulated prefix sums

    u     = s * Bu                     (s folded into B on host)
    E     = cumsum(T1 * u);  F = cumsum(T2 * u)     per complex part
    x_t   = sin(t th) * E_t + cos(t th) * F_t
    T1    = gamma*cos + sin;  T2 = cos - gamma*sin

Structure (keeps the 128 = 4 time-chunks x 32 states partition fold):
  - EVEN/ODD TIME SPLIT, done on the host: inpT columns are permuted per
    2048-chunk to [evens 1024 | odds 1024]; all tables pre-blocked to match.
    The DVE scan (2 cycles/col, no perf modes) then only runs over the
    1024 pair-sums P_j = y_{2j} + y_{2j+1}:
        S_{2j+1} = seed + cumsum(P)_j          (scan, half length)
        S_{2j}   = S_{2j+1} - y_{2j+1}         (aligned 2x tensor_tensor sub)
    halving the dominant scan cost.
  - u is evacuated from PSUM to bf16 SBUF by the ACT engine (4 separate
    e/o PSUM tiles so the even evacs start mid-load), so modulations and
    demodulations are all-bf16 full-width [128,2048] tensor_tensor ops in
    DVE 2x mode.
  - per-chunk carry sums come from ACT activation(Identity, accum_out=..)
    re-reading the pair-sums (off the DVE critical path); Wm matmul turns
    them into scan initial values (v3 mechanism).
  - DMA issue cost is ~0.6us PER dma_start on a HWDGE ring (measured): the
    issue stream is split across BOTH rings (sync + scalar), small tensors
    are packed into one transfer, and out-DMAs batch 2 evacs each.
  - projection/output as v3: two slabs out0/out1, host sums + un-permutes.
"""

import numpy as np

L, H, P = 8192, 128, 256
NCORES = 8
SLOC = P // NCORES          # states per core
FOLD = 4                    # time chunks folded into partitions
CL = L // FOLD              # 2048 free columns per partition row
HCL = CL // 2               # 1024 columns per even/odd half
NPART = FOLD * SLOC         # 128
JT = 512                    # matmul j-tile width

_CACHE: dict = {}


def _build_bass(split_waits=True):
    import concourse.bass as bass
    import concourse.mybir as mybir
    import concourse.tile as tile

    dt = mybir.dt.float32
    bt = mybir.dt.bfloat16
    Alu = mybir.AluOpType
    AF = mybir.ActivationFunctionType

    nc = bass.Bass(
        trn_type="TRN2",
        target_bir_lowering=False,
        debug=False,
        num_devices=NCORES,
    )

    inpT_d = nc.dram_tensor("inpT", [H, L], bt, kind="ExternalInput").ap()
    # packed: Bt [*,0:64] | Ctr [*,64:192] | Cti [*,192:320]
    BCC_d = nc.dram_tensor("BCC", [128, 320], bt, kind="ExternalInput").ap()
    Wm_d = nc.dram_tensor("Wm", [NPART, NPART], dt, kind="ExternalInput").ap()
    T1_d = nc.dram_tensor("T1blk", [NPART, CL], bt, kind="ExternalInput").ap()
    T2_d = nc.dram_tensor("T2blk", [NPART, CL], bt, kind="ExternalInput").ap()
    sin_d = nc.dram_tensor("sinblk", [NPART, CL], bt, kind="ExternalInput").ap()
    cos_d = nc.dram_tensor("cosblk", [NPART, CL], bt, kind="ExternalInput").ap()
    out0 = nc.dram_tensor("out0", [H, L], bt, kind="ExternalOutput").ap()
    out1 = nc.dram_tensor("out1", [H, L], bt, kind="ExternalOutput").ap()

    with tile.TileContext(nc) as tc:
        cpool = tc.alloc_tile_pool(name="const", bufs=1)
        big = tc.alloc_tile_pool(name="big", bufs=1)
        stage = tc.alloc_tile_pool(name="stage", bufs=4)
        pbu_re_pool = tc.alloc_tile_pool(name="pbu_re", bufs=1, space="PSUM")
        pbu_ro_pool = tc.alloc_tile_pool(name="pbu_ro", bufs=1, space="PSUM")
        pbu_ie_pool = tc.alloc_tile_pool(name="pbu_ie", bufs=1, space="PSUM")
        pbu_io_pool = tc.alloc_tile_pool(name="pbu_io", bufs=1, space="PSUM")

        # ---- loads; issue cost is ~0.6us per dma_start per ring, so the
        # stream is split across the two HWDGE rings (sync + scalar) ----
        # sync ring: the 8 inpT chunks (evens first), then T2/cos tables.
        inpT = big.tile([128, L], bt, tag="inpT")
        KB = L // 8  # 1024-col dma blocks; evens are dram blocks 0,2,4,6
        for k8 in (0, 2, 4, 6, 1, 3, 5, 7):
            nc.sync.dma_start(
                out=inpT[:, k8 * KB:(k8 + 1) * KB],
                in_=inpT_d[:, k8 * KB:(k8 + 1) * KB],
            )
        T2blk = big.tile([NPART, CL], bt, tag="T2blk")
        nc.sync.dma_start(out=T2blk[:], in_=T2_d)
        cosblk = big.tile([NPART, CL], bt, tag="cosblk")
        nc.sync.dma_start(out=cosblk[:], in_=cos_d)
        # scalar ring: small packed tensors + T1/sin tables.
        BCC = cpool.tile([128, 320], bt)
        nc.scalar.dma_start(out=BCC[:], in_=BCC_d)
        Wm = cpool.tile([NPART, NPART], dt)
        nc.scalar.dma_start(out=Wm[:], in_=Wm_d)
        T1blk = big.tile([NPART, CL], bt, tag="T1blk")
        nc.scalar.dma_start(out=T1blk[:], in_=T1_d)
        sinblk = big.tile([NPART, CL], bt, tag="sinblk")
        nc.scalar.dma_start(out=sinblk[:], in_=sin_d)
        Ctr = BCC[:, 2 * SLOC:2 * SLOC + H]
        Cti = BCC[:, 2 * SLOC + H:2 * SLOC + 2 * H]

        ones = cpool.tile([NPART, HCL], bt)
        nc.vector.memset(ones[:], 1.0)

        # ---- Bu matmuls; four separate psum tiles so the even-half evacs
        # can start while the odd chunks are still streaming ----
        pbu_re = pbu_re_pool.tile([NPART, HCL], dt, tag="bu_re")
        pbu_ro = pbu_ro_pool.tile([NPART, HCL], dt, tag="bu_ro")
        pbu_ie = pbu_ie_pool.tile([NPART, HCL], dt, tag="bu_ie")
        pbu_io = pbu_io_pool.tile([NPART, HCL], dt, tag="bu_io")
        bs_r = slice(0, SLOC)
        bs_i = slice(SLOC, 2 * SLOC)
        for half, (pr, pi) in ((0, (pbu_re, pbu_ie)), (1, (pbu_ro, pbu_io))):
            for c in range(FOLD):
                ps = slice(c * SLOC, (c + 1) * SLOC)
                for jh in range(2):
                    rhs = inpT[:, c * CL + half * HCL + jh * JT:
                               c * CL + half * HCL + (jh + 1) * JT]
                    js = slice(jh * JT, (jh + 1) * JT)
                    nc.tensor.matmul(pr[ps, js], BCC[:, bs_r], rhs,
                                     start=True, stop=True,
                                     tile_position=(0, c * SLOC))
                    nc.tensor.matmul(pi[ps, js], BCC[:, bs_i], rhs,
                                     start=True, stop=True,
                                     tile_position=(0, c * SLOC))

        # ---- ACT evac of u to bf16 SBUF ----
        u_r = big.tile([NPART, CL], bt, tag="u_r")
        u_i = big.tile([NPART, CL], bt, tag="u_i")
        nc.scalar.copy(u_r[:, 0:HCL], pbu_re[:])
        nc.scalar.copy(u_i[:, 0:HCL], pbu_ie[:])
        nc.scalar.copy(u_r[:, HCL:CL], pbu_ro[:])
        nc.scalar.copy(u_i[:, HCL:CL], pbu_io[:])

        # ---- modulations (tt 2x, full width) + pair-sums + carries ----
        A = cpool.tile([NPART, 4], dt)
        offs = cpool.tile([NPART, 4], dt)
        Pdum = big.tile([NPART, HCL], bt, tag="Pdum")
        Y = {}
        Pq = {}
        quants = [("1r", T1blk, u_r, 0), ("2r", T2blk, u_r, 1),
                  ("1i", T1blk, u_i, 2), ("2i", T2blk, u_i, 3)]

        def modpair(qi):
            q, T, u, ai = quants[qi]
            Yt = big.tile([NPART, CL], bt, tag=f"Y{q}")
            Pt = big.tile([NPART, HCL], bt, tag=f"P{q}")
            nc.vector.tensor_mul(Yt[:], T[:], u[:])
            nc.vector.tensor_add(Pt[:], Yt[:, 0:HCL], Yt[:, HCL:CL])
            # chunk sums on ACT, off the DVE chain
            nc.scalar.activation(Pdum[:], Pt[:], AF.Identity,
                                 accum_out=A[:, ai:ai + 1])
            Y[q] = Yt
            Pq[q] = Pt

        S = {}

        def scan(q, ai):
            St = big.tile([NPART, CL], bt, tag=f"S{q}")
            ini = (pbu_re[:, ai:ai + 1] if ai < 2
                   else offs[:, ai:ai + 1])
            bass.BassGpSimd.tensor_tensor_scan(
                nc.vector, St[:, HCL:CL], ones[:], Pq[q][:], ini,
                Alu.mult, Alu.add,
            )
            S[q] = St

        def fix(q):
            # S_even = S_odd - y_odd  (aligned, 2x)
            nc.vector.tensor_sub(S[q][:, 0:HCL], S[q][:, HCL:CL],
                                 Y[q][:, HCL:CL])

        modpair(0)                      # Y1r, P1r
        modpair(1)                      # Y2r, P2r
        nc.tensor.matmul(pbu_re[:, 0:2], Wm[:], A[:, 0:2],
                         start=True, stop=True)
        modpair(2)                      # Y1i, P1i  (fills DVE while carry lands)
        scan("1r", 0)
        modpair(3)                      # Y2i, P2i
        nc.tensor.matmul(pbu_ie[:, 0:2], Wm[:], A[:, 2:4],
                         start=True, stop=True)
        nc.scalar.copy(offs[:, 2:4], pbu_ie[:, 0:2])
        scan("2r", 1)
        fix("1r")
        fix("2r")
        m1r = big.tile([NPART, CL], bt, tag="m1r")
        m2r = big.tile([NPART, CL], bt, tag="m2r")
        x_r = big.tile([NPART, CL], bt, tag="x_r")
        nc.vector.tensor_mul(m1r[:], S["1r"][:], sinblk[:])
        nc.vector.tensor_mul(m2r[:], S["2r"][:], cosblk[:])
        nc.vector.tensor_add(x_r[:], m1r[:], m2r[:])
        scan("1i", 2)
        scan("2i", 3)
        fix("1i")
        fix("2i")
        m1i = big.tile([NPART, CL], bt, tag="m1i")
        m2i = big.tile([NPART, CL], bt, tag="m2i")
        x_i = big.tile([NPART, CL], bt, tag="x_i")
        nc.vector.tensor_mul(m1i[:], S["1i"][:], sinblk[:])
        nc.vector.tensor_mul(m2i[:], S["2i"][:], cosblk[:])
        nc.vector.tensor_add(x_i[:], m1i[:], m2i[:])

        pbu_io_pool.release()
        pbu_ie_pool.release()
        pbu_ro_pool.release()
        pbu_re_pool.release()
        po = tc.alloc_tile_pool(name="po", bufs=4, space="PSUM")

        # ---- projection slabs: out0 = Ctr@x_r (under the i chain),
        #      out1 = Cti@x_i (tail); host sums the slabs.
        # per chunk c the 2048 cols stay [evens 1024 | odds 1024] ----
        for slab, (Wt, x, outd) in enumerate(((Ctr, x_r, out0),
                                              (Cti, x_i, out1))):
            for c in range(FOLD):
                ps = slice(c * SLOC, (c + 1) * SLOC)
                st = stage.tile([128, CL], bt, tag="st")
                for hh in range(2):
                    pt = po.tile([128, 2 * JT], dt, tag="po")
                    for jh in range(2):
                        js = slice(hh * HCL + jh * JT,
                                   hh * HCL + (jh + 1) * JT)
                        nc.tensor.matmul(
                            pt[:, jh * JT:(jh + 1) * JT], Wt[ps, :],
                            x[ps, js], start=True, stop=True,
                            tile_position=(c * SLOC, 0),
                        )
                    # slab0 evacs ride the ACT engine under the i-chain;
                    # slab1 evacs land in the tail where DVE is free
                    if slab == 1 and hh == 0:
                        nc.vector.tensor_copy(st[:, hh * HCL:(hh + 1) * HCL],
                                              pt[:])
                    else:
                        nc.scalar.copy(st[:, hh * HCL:(hh + 1) * HCL], pt[:])
                nc.sync.dma_start(
                    out=outd[:, c * CL:(c + 1) * CL], in_=st[:])
        for p in (po, stage, big, cpool):
            p.release()
    if split_waits:
        _split_matmul_waits(nc, mybir)
    return nc


def _split_matmul_waits(nc, mybir):
    """Hardware instruction structs fit a limited number of embedded sync
    waits; move extra waits onto an inserted same-queue no-op."""
    caps = {"InstMatmult": 1}
    skip = {"InstNoOp", "InstAllEngineBarrier", "InstSync"}
    k = 0
    for bb in nc.main_func.blocks:
        insts = bb.instructions
        i = 0
        while i < len(insts):
            ins = insts[i]
            tn = type(ins).__name__
            if tn not in skip and ins.sync_info is not None:
                cap = caps.get(tn, 1)
                w = list(ins.sync_info.on_wait or [])
                if len(w) > cap:
                    for wj in w[:-cap]:
                        nop = mybir.InstNoOp(
                            name=f"I-mmdep-{k}",
                            engine=ins.engine,
                            ins=[],
                            outs=[],
                            sync_info=mybir.SyncInfo(
                                on_wait=[wj], on_update=[]
                            ),
                        )
                        k += 1
                        insts.insert(i, nop)
                        i += 1
                    ins.sync_info = mybir.SyncInfo(
                        on_wait=w[-cap:], on_update=ins.sync_info.on_update
                    )
            i += 1


def _eo_permute(a):
    """per 2048-col chunk: natural t' order -> [evens 1024 | odds 1024]."""
    r, n = a.shape
    nch = n // CL
    return np.ascontiguousarray(
        a.reshape(r, nch, CL // 2, 2).transpose(0, 1, 3, 2).reshape(r, n))


def _eo_unpermute(a):
    r, n = a.shape
    nch = n // CL
    return np.ascontiguousarray(
        a.reshape(r, nch, 2, CL // 2).transpose(0, 1, 3, 2).reshape(r, n))


def _host_prep(inputs):
    import ml_dtypes
    bf16 = ml_dtypes.bfloat16
    f32 = np.float32

    inp32 = np.asarray(inputs["input_sequence"], np.float32)
    inpT = _eo_permute(np.ascontiguousarray(inp32.T)).astype(bf16)
    A = np.maximum(np.asarray(inputs["A_diag_raw"], np.float64), 0.0)
    s = 1.0 / (1.0 + np.exp(-np.asarray(inputs["steps_raw"], np.float64)))
    Br = np.asarray(inputs["B_real"], np.float64)
    Bi = np.asarray(inputs["B_img"], np.float64)
    Cr = np.asarray(inputs["C_real"], np.float64)
    Ci = np.asarray(inputs["C_img"], np.float64)

    costh = 1.0 - s * s * A / 2.0
    sinth = np.sqrt(np.maximum(1.0 - costh * costh, 1e-300))
    theta = np.arctan2(sinth, costh)
    gamma = (s - s * s * A / 2.0) / sinth

    q = np.arange(NPART)
    Wm = ((q[:, None] % SLOC == q[None, :] % SLOC)
          & (q[:, None] // SLOC < q[None, :] // SLOC)).astype(f32)

    tvec = np.arange(CL, dtype=np.float64)
    twopi = 2.0 * np.pi

    in_maps = []
    for k in range(NCORES):
        sl = slice(k * SLOC, (k + 1) * SLOC)
        th = theta[sl]
        gm = gamma[sl]
        BCC = np.empty((128, 320), bf16)
        BCC[:, 0:SLOC] = (s[sl, None] * Br[sl]).T.astype(bf16)
        BCC[:, SLOC:2 * SLOC] = (s[sl, None] * Bi[sl]).T.astype(bf16)
        BCC[:, 2 * SLOC:2 * SLOC + H] = np.tile(
            Cr[:, sl].T, (FOLD, 1)).astype(bf16)
        BCC[:, 2 * SLOC + H:] = np.tile(
            -Ci[:, sl].T, (FOLD, 1)).astype(bf16)

        # tables per partition q = c*SLOC + s at global time t = c*CL + j
        ang = np.empty((NPART, CL), np.float64)
        for c in range(FOLD):
            ang[c * SLOC:(c + 1) * SLOC] = np.mod(
                (c * CL + tvec)[None, :] * th[:, None], twopi)
        sinA = np.sin(ang)
        cosA = np.cos(ang)
        gq = np.tile(gm, FOLD)[:, None]
        T1 = gq * cosA + sinA
        T2 = cosA - gq * sinA

        m = {"inpT": inpT, "BCC": BCC, "Wm": Wm}
        for nm, tb in (("T1blk", T1), ("T2blk", T2),
                       ("sinblk", sinA), ("cosblk", cosA)):
            m[nm] = _eo_permute(np.ascontiguousarray(tb)).astype(bf16)
        in_maps.append(m)
    return in_maps


LAST_RESULTS = None


def kernel(**inputs) -> np.ndarray:
    global LAST_RESULTS
    from concourse.bass_utils import run_bass_kernel_spmd

    if "nc" not in _CACHE:
        _CACHE["nc"] = _build_bass()
    nc = _CACHE["nc"]

    in_maps = _host_prep(inputs)
    res = run_bass_kernel_spmd(nc, in_maps, core_ids=list(range(NCORES)))
    LAST_RESULTS = res
    part = np.zeros((H, L), np.float32)
    for r in res.results:
        part += np.asarray(r["out0"], np.float32)
        part += np.asarray(r["out1"], np.float32)
    out = np.ascontiguousarray(_eo_unpermute(part).T)
    out += (np.asarray(inputs["input_sequence"], np.float32)
            * np.asarray(inputs["D"], np.float32)[None, :])
    return out


# revision 12
# speedup vs baseline: 1.1461x; 1.0763x over previous
"""LinOSS layer Trainium2 kernel, v4.2.

Math (same closed form as v3): the per-state 2x2 recurrence has eigenvalues
e^{+-i theta}; the scanned state collapses to rank-2 modulated prefix sums

    u     = s * Bu                     (s folded into B on host)
    E     = cumsum(T1 * u);  F = cumsum(T2 * u)     per complex part
    x_t   = sin(t th) * E_t + cos(t th) * F_t
    T1    = gamma*cos + sin;  T2 = cos - gamma*sin

Structure (keeps the 128 = 4 time-chunks x 32 states partition fold):
  - EVEN/ODD TIME SPLIT, done on the host: inpT columns are permuted per
    2048-chunk to [evens 1024 | odds 1024]; all tables pre-blocked to match.
    The DVE scan (2 cycles/col, no perf modes) then only runs over the
    1024 pair-sums P_j = y_{2j} + y_{2j+1}:
        S_{2j+1} = seed + cumsum(P)_j          (scan, half length)
        S_{2j}   = S_{2j+1} - y_{2j+1}         (aligned 2x tensor_tensor sub)
    halving the dominant scan cost.
  - u is evacuated from PSUM to bf16 SBUF by the ACT engine (4 separate
    e/o PSUM tiles so the even evacs start mid-load), so modulations and
    demodulations are all-bf16 full-width [128,2048] tensor_tensor ops in
    DVE 2x mode.
  - per-chunk carry sums come from ACT activation(Identity, accum_out=..)
    re-reading the pair-sums (off the DVE critical path); Wm matmul turns
    them into scan initial values (v3 mechanism).
  - DMA issue cost is ~0.6us PER dma_start on a HWDGE ring (measured): the
    issue stream is split across BOTH rings (sync + scalar), small tensors
    are packed into one transfer, and out-DMAs batch 2 evacs each.
  - projection/output as v3: two slabs out0/out1, host sums + un-permutes.
"""

import numpy as np

L, H, P = 8192, 128, 256
NCORES = 8
SLOC = P // NCORES          # states per core
FOLD = 4                    # time chunks folded into partitions
CL = L // FOLD              # 2048 free columns per partition row
HCL = CL // 2               # 1024 columns per even/odd half
NPART = FOLD * SLOC         # 128
JT = 512                    # matmul j-tile width

_CACHE: dict = {}


def _build_bass(split_waits=True):
    import concourse.bass as bass
    import concourse.mybir as mybir
    import concourse.tile as tile

    dt = mybir.dt.float32
    bt = mybir.dt.bfloat16
    Alu = mybir.AluOpType
    AF = mybir.ActivationFunctionType

    nc = bass.Bass(
        trn_type="TRN2",
        target_bir_lowering=False,
        debug=False,
        num_devices=NCORES,
    )

    inpT_d = nc.dram_tensor("inpT", [H, L], bt, kind="ExternalInput").ap()
    # packed: Bt [*,0:64] | Ctr [*,64:192] | Cti [*,192:320]
    BCC_d = nc.dram_tensor("BCC", [128, 320], bt, kind="ExternalInput").ap()
    Wm_d = nc.dram_tensor("Wm", [NPART, NPART], dt, kind="ExternalInput").ap()
    T1_d = nc.dram_tensor("T1blk", [NPART, CL], bt, kind="ExternalInput").ap()
    T2_d = nc.dram_tensor("T2blk", [NPART, CL], bt, kind="ExternalInput").ap()
    sin_d = nc.dram_tensor("sinblk", [NPART, CL], bt, kind="ExternalInput").ap()
    cos_d = nc.dram_tensor("cosblk", [NPART, CL], bt, kind="ExternalInput").ap()
    out0 = nc.dram_tensor("out0", [H, L], bt, kind="ExternalOutput").ap()
    out1 = nc.dram_tensor("out1", [H, L], bt, kind="ExternalOutput").ap()

    with tile.TileContext(nc) as tc:
        cpool = tc.alloc_tile_pool(name="const", bufs=1)
        big = tc.alloc_tile_pool(name="big", bufs=1)
        stage = tc.alloc_tile_pool(name="stage", bufs=4)
        pbu_re_pool = tc.alloc_tile_pool(name="pbu_re", bufs=1, space="PSUM")
        pbu_ro_pool = tc.alloc_tile_pool(name="pbu_ro", bufs=1, space="PSUM")
        pbu_ie_pool = tc.alloc_tile_pool(name="pbu_ie", bufs=1, space="PSUM")
        pbu_io_pool = tc.alloc_tile_pool(name="pbu_io", bufs=1, space="PSUM")

        # ---- loads; issue cost is ~0.6us per dma_start per ring, so the
        # stream is split across the two HWDGE rings (sync + scalar) ----
        # sync ring: the 8 inpT chunks (evens first), then T2/cos tables.
        inpT = big.tile([128, L], bt, tag="inpT")
        KB = L // 8  # 1024-col dma blocks; evens are dram blocks 0,2,4,6
        for k8 in (0, 2, 4, 6, 1, 3, 5, 7):
            nc.sync.dma_start(
                out=inpT[:, k8 * KB:(k8 + 1) * KB],
                in_=inpT_d[:, k8 * KB:(k8 + 1) * KB],
            )
        # tables are WAR-gated on inpT progress (gpsimd dummy reading an
        # inpT block + writing a dummy slot of the table tile) so the
        # fair-shared DMA bandwidth stays on the critical input chunks
        def gate(name, gate_k8):
            dummy = big.tile([NPART, CL], bt, tag=name)
            gd = cpool.tile([1, 8], dt, tag=f"g_{name}")
            nc.gpsimd.memset(dummy[0:1, 0:8], 0.0)
            nc.gpsimd.tensor_tensor(
                gd[:], dummy[0:1, 0:8],
                inpT[0:1, gate_k8 * KB:gate_k8 * KB + 8], Alu.add)
            real = big.tile([NPART, CL], bt, tag=name)
            return real

        T2blk = gate("T2blk", 6)       # after 4th even block
        nc.sync.dma_start(out=T2blk[:], in_=T2_d)
        # scalar ring: small packed tensors + T1/sin/cos tables.
        BCC = cpool.tile([128, 320], bt)
        nc.scalar.dma_start(out=BCC[:], in_=BCC_d)
        Wm = cpool.tile([NPART, NPART], dt)
        nc.scalar.dma_start(out=Wm[:], in_=Wm_d)
        T1blk = gate("T1blk", 4)       # after 3rd even block
        nc.scalar.dma_start(out=T1blk[:], in_=T1_d)
        sinblk = gate("sinblk", 3)     # after 2nd odd block
        cosblk = gate("cosblk", 3)
        Ctr = BCC[:, 2 * SLOC:2 * SLOC + H]
        Cti = BCC[:, 2 * SLOC + H:2 * SLOC + 2 * H]

        ones = cpool.tile([NPART, HCL], bt)
        nc.vector.memset(ones[:], 1.0)

        # ---- Bu matmuls; four separate psum tiles so the even-half evacs
        # can start while the odd chunks are still streaming ----
        pbu_re = pbu_re_pool.tile([NPART, HCL], dt, tag="bu_re")
        pbu_ro = pbu_ro_pool.tile([NPART, HCL], dt, tag="bu_ro")
        pbu_ie = pbu_ie_pool.tile([NPART, HCL], dt, tag="bu_ie")
        pbu_io = pbu_io_pool.tile([NPART, HCL], dt, tag="bu_io")
        bs_r = slice(0, SLOC)
        bs_i = slice(SLOC, 2 * SLOC)
        for half, (pr, pi) in ((0, (pbu_re, pbu_ie)), (1, (pbu_ro, pbu_io))):
            for c in range(FOLD):
                ps = slice(c * SLOC, (c + 1) * SLOC)
                for jh in range(2):
                    rhs = inpT[:, c * CL + half * HCL + jh * JT:
                               c * CL + half * HCL + (jh + 1) * JT]
                    js = slice(jh * JT, (jh + 1) * JT)
                    nc.tensor.matmul(pr[ps, js], BCC[:, bs_r], rhs,
                                     start=True, stop=True,
                                     tile_position=(0, c * SLOC))
                    nc.tensor.matmul(pi[ps, js], BCC[:, bs_i], rhs,
                                     start=True, stop=True,
                                     tile_position=(0, c * SLOC))

        # ---- ACT evac of u to bf16 SBUF; even halves first, then the
        # gated sin/cos dma issues, then odd halves (queue-order matters:
        # a gated dma_start blocks later entries of its issuing ring) ----
        u_r = big.tile([NPART, CL], bt, tag="u_r")
        u_i = big.tile([NPART, CL], bt, tag="u_i")
        nc.scalar.copy(u_r[:, 0:HCL], pbu_re[:])
        nc.scalar.copy(u_i[:, 0:HCL], pbu_ie[:])
        nc.scalar.dma_start(out=sinblk[:], in_=sin_d)
        nc.scalar.dma_start(out=cosblk[:], in_=cos_d)
        nc.scalar.copy(u_r[:, HCL:CL], pbu_ro[:])
        nc.scalar.copy(u_i[:, HCL:CL], pbu_io[:])

        # ---- modulations (tt 2x) + pair-sums + carries.  The even-half
        # mods run during the odd input DMAs; odd halves + pair sums chase
        # the odd evacs ----
        A = cpool.tile([NPART, 4], dt)
        offs = cpool.tile([NPART, 4], dt)
        Pdum = big.tile([NPART, HCL], bt, tag="Pdum")
        Y = {}
        Pq = {}
        quants = [("1r", T1blk, u_r, 0), ("2r", T2blk, u_r, 1),
                  ("1i", T1blk, u_i, 2), ("2i", T2blk, u_i, 3)]
        for q, T, u, ai in quants:
            Yt = big.tile([NPART, CL], bt, tag=f"Y{q}")
            nc.vector.tensor_mul(Yt[:, 0:HCL], T[:, 0:HCL], u[:, 0:HCL])
            Y[q] = Yt

        def modpair(qi):
            q, T, u, ai = quants[qi]
            Yt = Y[q]
            Pt = big.tile([NPART, HCL], bt, tag=f"P{q}")
            nc.vector.tensor_mul(Yt[:, HCL:CL], T[:, HCL:CL], u[:, HCL:CL])
            nc.vector.tensor_add(Pt[:], Yt[:, 0:HCL], Yt[:, HCL:CL])
            # chunk sums on ACT, off the DVE chain
            nc.scalar.activation(Pdum[:], Pt[:], AF.Identity,
                                 accum_out=A[:, ai:ai + 1])
            Pq[q] = Pt

        S = {}

        def scan(q, ai):
            St = big.tile([NPART, CL], bt, tag=f"S{q}")
            ini = (pbu_re[:, ai:ai + 1] if ai < 2
                   else offs[:, ai:ai + 1])
            bass.BassGpSimd.tensor_tensor_scan(
                nc.vector, St[:, HCL:CL], ones[:], Pq[q][:], ini,
                Alu.mult, Alu.add,
            )
            S[q] = St

        def fix(q):
            # S_even = S_odd - y_odd  (aligned, 2x)
            nc.vector.tensor_sub(S[q][:, 0:HCL], S[q][:, HCL:CL],
                                 Y[q][:, HCL:CL])

        modpair(0)                      # Y1r odd, P1r
        modpair(1)                      # Y2r odd, P2r
        nc.tensor.matmul(pbu_re[:, 0:2], Wm[:], A[:, 0:2],
                         start=True, stop=True)
        modpair(2)                      # Y1i, P1i  (fills DVE while carry lands)
        scan("1r", 0)
        modpair(3)                      # Y2i, P2i
        nc.tensor.matmul(pbu_ie[:, 0:2], Wm[:], A[:, 2:4],
                         start=True, stop=True)
        nc.scalar.copy(offs[:, 2:4], pbu_ie[:, 0:2])
        scan("2r", 1)
        fix("1r")
        fix("2r")
        m1r = big.tile([NPART, CL], bt, tag="m1r")
        m2r = big.tile([NPART, CL], bt, tag="m2r")
        x_r = big.tile([NPART, CL], bt, tag="x_r")
        nc.vector.tensor_mul(m1r[:], S["1r"][:], sinblk[:])
        nc.vector.tensor_mul(m2r[:], S["2r"][:], cosblk[:])
        nc.vector.tensor_add(x_r[:, 0:HCL], m1r[:, 0:HCL], m2r[:, 0:HCL])
        nc.vector.tensor_add(x_r[:, HCL:CL], m1r[:, HCL:CL], m2r[:, HCL:CL])
        scan("1i", 2)
        scan("2i", 3)
        fix("1i")
        fix("2i")
        m1i = big.tile([NPART, CL], bt, tag="m1i")
        m2i = big.tile([NPART, CL], bt, tag="m2i")
        x_i = big.tile([NPART, CL], bt, tag="x_i")
        nc.vector.tensor_mul(m1i[:], S["1i"][:], sinblk[:])
        nc.vector.tensor_mul(m2i[:], S["2i"][:], cosblk[:])
        nc.vector.tensor_add(x_i[:, 0:HCL], m1i[:, 0:HCL], m2i[:, 0:HCL])
        nc.vector.tensor_add(x_i[:, HCL:CL], m1i[:, HCL:CL], m2i[:, HCL:CL])

        pbu_io_pool.release()
        pbu_ie_pool.release()
        pbu_ro_pool.release()
        pbu_re_pool.release()
        po = tc.alloc_tile_pool(name="po", bufs=4, space="PSUM")

        # ---- projection slabs: out0 = Ctr@x_r (under the i chain),
        #      out1 = Cti@x_i (tail); host sums the slabs.
        # per chunk c the 2048 cols stay [evens 1024 | odds 1024] ----
        for slab, (Wt, x, outd) in enumerate(((Ctr, x_r, out0),
                                              (Cti, x_i, out1))):
            for c in range(FOLD):
                ps = slice(c * SLOC, (c + 1) * SLOC)
                st = stage.tile([128, CL], bt, tag="st")
                for hh in range(2):
                    pt = po.tile([128, 2 * JT], dt, tag="po")
                    for jh in range(2):
                        js = slice(hh * HCL + jh * JT,
                                   hh * HCL + (jh + 1) * JT)
                        nc.tensor.matmul(
                            pt[:, jh * JT:(jh + 1) * JT], Wt[ps, :],
                            x[ps, js], start=True, stop=True,
                            tile_position=(c * SLOC, 0),
                        )
                    # slab0 evacs ride the ACT engine under the i-chain;
                    # slab1 evacs land in the tail where DVE is free
                    if slab == 1 and hh == 0:
                        nc.vector.tensor_copy(st[:, hh * HCL:(hh + 1) * HCL],
                                              pt[:])
                    else:
                        nc.scalar.copy(st[:, hh * HCL:(hh + 1) * HCL], pt[:])
                nc.sync.dma_start(
                    out=outd[:, c * CL:(c + 1) * CL], in_=st[:])
        for p in (po, stage, big, cpool):
            p.release()
    if split_waits:
        _split_matmul_waits(nc, mybir)
    return nc


def _split_matmul_waits(nc, mybir):
    """Hardware instruction structs fit a limited number of embedded sync
    waits; move extra waits onto an inserted same-queue no-op."""
    caps = {"InstMatmult": 1}
    skip = {"InstNoOp", "InstAllEngineBarrier", "InstSync"}
    k = 0
    for bb in nc.main_func.blocks:
        insts = bb.instructions
        i = 0
        while i < len(insts):
            ins = insts[i]
            tn = type(ins).__name__
            if tn not in skip and ins.sync_info is not None:
                cap = caps.get(tn, 1)
                w = list(ins.sync_info.on_wait or [])
                if len(w) > cap:
                    for wj in w[:-cap]:
                        nop = mybir.InstNoOp(
                            name=f"I-mmdep-{k}",
                            engine=ins.engine,
                            ins=[],
                            outs=[],
                            sync_info=mybir.SyncInfo(
                                on_wait=[wj], on_update=[]
                            ),
                        )
                        k += 1
                        insts.insert(i, nop)
                        i += 1
                    ins.sync_info = mybir.SyncInfo(
                        on_wait=w[-cap:], on_update=ins.sync_info.on_update
                    )
            i += 1


def _eo_permute(a):
    """per 2048-col chunk: natural t' order -> [evens 1024 | odds 1024]."""
    r, n = a.shape
    nch = n // CL
    return np.ascontiguousarray(
        a.reshape(r, nch, CL // 2, 2).transpose(0, 1, 3, 2).reshape(r, n))


def _eo_unpermute(a):
    r, n = a.shape
    nch = n // CL
    return np.ascontiguousarray(
        a.reshape(r, nch, 2, CL // 2).transpose(0, 1, 3, 2).reshape(r, n))


def _host_prep(inputs):
    import ml_dtypes
    bf16 = ml_dtypes.bfloat16
    f32 = np.float32

    inp32 = np.asarray(inputs["input_sequence"], np.float32)
    inpT = _eo_permute(np.ascontiguousarray(inp32.T)).astype(bf16)
    A = np.maximum(np.asarray(inputs["A_diag_raw"], np.float64), 0.0)
    s = 1.0 / (1.0 + np.exp(-np.asarray(inputs["steps_raw"], np.float64)))
    Br = np.asarray(inputs["B_real"], np.float64)
    Bi = np.asarray(inputs["B_img"], np.float64)
    Cr = np.asarray(inputs["C_real"], np.float64)
    Ci = np.asarray(inputs["C_img"], np.float64)

    costh = 1.0 - s * s * A / 2.0
    sinth = np.sqrt(np.maximum(1.0 - costh * costh, 1e-300))
    theta = np.arctan2(sinth, costh)
    gamma = (s - s * s * A / 2.0) / sinth

    q = np.arange(NPART)
    Wm = ((q[:, None] % SLOC == q[None, :] % SLOC)
          & (q[:, None] // SLOC < q[None, :] // SLOC)).astype(f32)

    tvec = np.arange(CL, dtype=np.float64)
    twopi = 2.0 * np.pi

    in_maps = []
    for k in range(NCORES):
        sl = slice(k * SLOC, (k + 1) * SLOC)
        th = theta[sl]
        gm = gamma[sl]
        BCC = np.empty((128, 320), bf16)
        BCC[:, 0:SLOC] = (s[sl, None] * Br[sl]).T.astype(bf16)
        BCC[:, SLOC:2 * SLOC] = (s[sl, None] * Bi[sl]).T.astype(bf16)
        BCC[:, 2 * SLOC:2 * SLOC + H] = np.tile(
            Cr[:, sl].T, (FOLD, 1)).astype(bf16)
        BCC[:, 2 * SLOC + H:] = np.tile(
            -Ci[:, sl].T, (FOLD, 1)).astype(bf16)

        # tables per partition q = c*SLOC + s at global time t = c*CL + j
        ang = np.empty((NPART, CL), np.float64)
        for c in range(FOLD):
            ang[c * SLOC:(c + 1) * SLOC] = np.mod(
                (c * CL + tvec)[None, :] * th[:, None], twopi)
        sinA = np.sin(ang)
        cosA = np.cos(ang)
        gq = np.tile(gm, FOLD)[:, None]
        T1 = gq * cosA + sinA
        T2 = cosA - gq * sinA

        m = {"inpT": inpT, "BCC": BCC, "Wm": Wm}
        for nm, tb in (("T1blk", T1), ("T2blk", T2),
                       ("sinblk", sinA), ("cosblk", cosA)):
            m[nm] = _eo_permute(np.ascontiguousarray(tb)).astype(bf16)
        in_maps.append(m)
    return in_maps


LAST_RESULTS = None


def kernel(**inputs) -> np.ndarray:
    global LAST_RESULTS
    from concourse.bass_utils import run_bass_kernel_spmd

    if "nc" not in _CACHE:
        _CACHE["nc"] = _build_bass()
    nc = _CACHE["nc"]

    in_maps = _host_prep(inputs)
    res = run_bass_kernel_spmd(nc, in_maps, core_ids=list(range(NCORES)))
    LAST_RESULTS = res
    part = np.zeros((H, L), np.float32)
    for r in res.results:
        part += np.asarray(r["out0"], np.float32)
        part += np.asarray(r["out1"], np.float32)
    out = np.ascontiguousarray(_eo_unpermute(part).T)
    out += (np.asarray(inputs["input_sequence"], np.float32)
            * np.asarray(inputs["D"], np.float32)[None, :])
    return out
